# revision 27
# baseline (speedup 1.0000x reference)
"""DCGRU cell (DCRNN) Trainium2 Bass kernel.

Strategy (see spec sharding_hint): data-parallel over batch B=64 across 8
NeuronCores (8 batches per core); supports + gconv weights replicated.

Math restructuring (validated in numpy against the jax reference):
  reference diffusion xs = [x0, S0@x0, 2*S0^2@x0 - x0, S1@S0@x0, 2*S1^2@S0@x0 - S0@x0]
  -> raw chain     ys = [y0, y1=S0@y0, y2=S0@y1, y3=S1@y1, y4=S1@y3]
  with the 2a-b combinations folded into the projection weights on the host:
  What = [W0-W2, W1-W4, 2*W2, W3, 2*W4] (Wm = rows insz*5+m of the gconv W).

Quantization (validated in numpy: rel err ~2e-4 vs fp32 reference):
  The diffusion chain runs in fp8e4 with MatmulPerfMode.DoubleRow (2
  k-subtiles per matmul, 2x PE throughput vs fp32r/bf16).  S entries are
  ~2.4e-4 (below fp8 subnormal range) so supports are scaled by 2^11 on the
  host; hop outputs are descaled and restored to fp8 with value scale 2^5
  (diffused stds ~0.015) by a fused scaled-copy on the ACT engine.  The 2^-5
  storage scale is folded into the projection weights per diffusion matrix.
  The projection runs in bf16 (weights, YT spills, m=0 operands); the final
  output terms are diluted ~50x relative to the diffusion values, so the
  fp8 chain error contributes only ~2e-4 to the final relative error.

Per-core device layout:
  Diffusion state X [N, 528] fp8 in SBUF, columns c = b*64+u (hx part,
  b=0..7) then 512 + b*2 + j (input part).  Hops are PE DoubleRow matmuls
  out[nb-block, c] += ST_tile[2 k-subtiles].T @ X[2 k-subtiles, c] with
  host-pretransposed, block-packed fp8 supports streamed from HBM.
  After each hop the result is transposed on PE (fp8, 128x128 chunks) and
  spilled to DRAM as YT [528-ish, N] bf16 so the projection can contract
  over features with the feature dim on partitions.  Projection:
  ZT_b[out,n] = sum_m What_m.T @ YT_m[b-rows, n] accumulated in PSUM (bf16
  operands), fused bias+sigmoid/tanh on ACT, gate arithmetic on DVE in
  fp32, all in [units, n] layout; host un-transposes the final output
  during unsharding.
"""

import os
from contextlib import ExitStack

import numpy as np
import ml_dtypes

import concourse.bacc as bacc
import concourse.mybir as mybir
import concourse.tile as tile
from concourse.bass_utils import run_bass_kernel_spmd
from concourse.masks import make_identity

F32 = mybir.dt.float32
F32R = mybir.dt.float32r
BF16 = mybir.dt.bfloat16
F8 = mybir.dt.float8e4
DR = mybir.MatmulPerfMode.DoubleRow

NP_F8 = ml_dtypes.float8_e4m3
NP_BF16 = ml_dtypes.bfloat16

S_SCALE = 2.0**11  # host: supports scaled into fp8 normal range
Y_SCALE = 2.0**5  # stored scale of diffused chain values (stds ~0.015)
# ACT descale on the psum->fp8 hop copy: hop 0 input is unscaled (y0),
# hops >=1 input carries Y_SCALE.
COPY_SCALE_H0 = Y_SCALE / S_SCALE
COPY_SCALE = Y_SCALE / (S_SCALE * Y_SCALE)


def _r(ap):
    return ap.bitcast(F32R)

NCORES = 8
B = 64
BLOC = B // NCORES  # 8
IN_DIM = 2
UNITS = 64
CHX = BLOC * UNITS  # 512
C = CHX + BLOC * IN_DIM  # 528
CIN = BLOC * IN_DIM  # 16
CH = C // 2  # 264 (psum free-dim split)


def _build_nc(N):
    """Build the per-core Bass program (SPMD; same NEFF on all 8 cores)."""
    NB = N // 128  # row blocks (32 at full size)
    PCH = min(2048, N)  # phase-P n-chunk held in SBUF
    NHALF = N // PCH
    NFC = PCH // 512  # 512-wide proj chunks per PCH

    nc = bacc.Bacc("TRN2", target_bir_lowering=False, debug=False)

    # ---- external I/O ----
    x0pm = nc.dram_tensor("x0pm", [128, NB * C], F8, kind="ExternalInput").ap()
    # pair-interleaved transposed supports: stb[s, np, kp, kb*256 + j*128 + m]
    # = S[(2*np+j)*128 + m, kb*128 + kp] * S_SCALE
    stb = nc.dram_tensor(
        "stb", [2, NB // 2, 128, NB * 256], F8, kind="ExternalInput"
    ).ap()
    xint = nc.dram_tensor("xint", [CIN, N], BF16, kind="ExternalInput").ap()
    hxt = nc.dram_tensor("hxt", [BLOC, UNITS, N], F32, kind="ExternalInput").ap()
    hxtb = nc.dram_tensor("hxtb", [BLOC, UNITS, N], BF16, kind="ExternalInput").ap()
    wfn = nc.dram_tensor("wfn", [3, 128, 128], BF16, kind="ExternalInput").ap()
    wg = nc.dram_tensor("wg", [3, 128, 64], BF16, kind="ExternalInput").ap()
    bfn = nc.dram_tensor("bfn", [128, 1], F32, kind="ExternalInput").ap()
    bg = nc.dram_tensor("bg", [64, 1], F32, kind="ExternalInput").ap()
    outt = nc.dram_tensor("outt", [BLOC, UNITS, N], F32, kind="ExternalOutput").ap()

    with tile.TileContext(nc) as tc, ExitStack() as ctx:
        # ---- persistent pools ----
        const = ctx.enter_context(tc.tile_pool(name="const", bufs=1))
        dram = ctx.enter_context(tc.tile_pool(name="dram", bufs=1, space="DRAM"))

        ident8 = const.tile([128, 128], F8, name="ident8")
        make_identity(nc, ident8)
        # packed projection weights per gconv: w0 = [m0 rows (66) + in-rows of
        # m1..4 (8)] = 74 rows; w1 = hx rows of m1,m2 (128); w2 = m3,m4 (128)
        w_sb = {}
        for g, wsrc, D in ((0, wfn, 128), (1, wg, 64)):
            tiles = []
            for t, rows in ((0, 74), (1, 128), (2, 128)):
                wt = const.tile([rows, D], BF16, name=f"w{g}_{t}")
                nc.sync.dma_start(wt, wsrc[t, 0:rows, 0:D])
                tiles.append(wt)
            w_sb[g] = tiles
        bfn_sb = const.tile([128, 1], F32, name="bfn_sb")
        nc.sync.dma_start(bfn_sb, bfn)
        bg_sb = const.tile([64, 1], F32, name="bg_sb")
        nc.sync.dma_start(bg_sb, bg)
        # DRAM scratch (bf16): per-gconv packed transposed diffusion results.
        # ytb[g] rows r = b*256 + (m-1)*64 + u (hx rows of hops m=1..4, packed
        # so the projection loads 128-row slabs); inb rows r = b*8 + (m-1)*2
        # + j (input-feature rows of hops m=1..4, shared by both gconvs).
        if os.environ.get("DCGRU_DEBUG", "0") == "1":
            ytb = [
                nc.dram_tensor(f"ytb{g}", [BLOC * 256, N], BF16, kind="ExternalOutput").ap()
                for g in range(2)
            ]
            inb = nc.dram_tensor("inb", [BLOC * 8, N], BF16, kind="ExternalOutput").ap()
            yt0p = nc.dram_tensor("yt0p", [CHX, N], BF16, kind="ExternalOutput").ap()
        else:
            ytb = [
                dram.tile([BLOC * 256, N], BF16, name=f"ytb{g}", tag=f"ytb{g}")
                for g in range(2)
            ]
            inb = dram.tile([BLOC * 8, N], BF16, name="inb", tag="inb")
            yt0p = dram.tile([CHX, N], BF16, name="yt0p", tag="yt0p")
        u_d = dram.tile([BLOC, UNITS, N], F32, name="u_d", tag="u_d")

        def diffusion(g):
            """4 hops; X0 loaded from DRAM (x0pm for g=0, yt0p^T for g=1)."""
            with (
                tc.tile_pool(name=f"ybuf{g}", bufs=1) as yp,
                tc.tile_pool(name=f"st{g}", bufs=2) as stp,
                tc.tile_pool(name=f"dps{g}", bufs=2, space="PSUM") as dps,
                tc.tile_pool(name=f"ips{g}", bufs=2, space="PSUM") as ips,
                tc.tile_pool(name=f"tps{g}", bufs=2, space="PSUM") as tps,
                tc.tile_pool(name=f"yts{g}", bufs=3) as ytsp,
            ):
                bufA = yp.tile([128, NB * C], F8, name=f"bufA{g}", tag="bufA")
                bufB = yp.tile([128, NB * C], F8, name=f"bufB{g}", tag="bufB")
                if g == 0:
                    q4 = NB * C // 4
                    for q in range(4):
                        nc.sync.dma_start(
                            bufA[:, q * q4 : (q + 1) * q4],
                            x0pm[:, q * q4 : (q + 1) * q4],
                        )
                else:
                    # x0' = r*hx lives transposed in yt0p [512, N]; XBAR
                    # DMA-transpose each node block then cast bf16 -> fp8
                    for kb in range(NB):
                        xb = ytsp.tile([128, CHX], BF16, name="xb", tag="xb")
                        nc.sync.dma_start_transpose(
                            xb, yt0p[:, kb * 128 : (kb + 1) * 128]
                        )
                        nc.vector.tensor_copy(
                            bufA[:, kb * C : kb * C + CHX], xb
                        )

                # gconv2 skips the 16 input columns entirely: their diffusion
                # is identical to gconv1's, so phase P reuses g1's spills.
                # packed spill views (see ytb/inb row layout comments)
                ytbv = ytb[g].rearrange(
                    "(jj bs mm u) n -> bs u jj mm n", jj=4, bs=2, mm=4, u=64
                )

                def hop(src, dst, s_idx, m, cscale):
                    src3 = src.rearrange("p (k c) -> p k c", c=C)

                    def load_pair(np_):
                        slab = stp.tile(
                            [128, NB * 256], F8, name=f"slab{g}", tag="slab"
                        )
                        nc.sync.dma_start(slab, stb[s_idx, np_])
                        return slab

                    def compute_block(slab, np_, bsel):
                        # main hx chain: single 512-wide psum, 16 DoubleRow
                        # matmuls (2 k-subtiles each)
                        nb = 2 * np_ + bsel
                        slabM = slab.rearrange(
                            "p (kb j c) -> p kb j c", j=2, c=128
                        )
                        pa = dps.tile([128, CHX], F32, name=f"pa{g}", tag="pa")
                        for ki in range(NB // 2):
                            nc.tensor.matmul(
                                pa,
                                slabM[:, 2 * ki : 2 * ki + 2, bsel, :],
                                src3[:, 2 * ki : 2 * ki + 2, 0:CHX],
                                start=(ki == 0),
                                stop=(ki == NB // 2 - 1),
                                perf_mode=DR,
                            )
                        # fused descale + fp32->fp8 store on ACT
                        nc.scalar.activation(
                            dst[:, nb * C : nb * C + CHX],
                            pa,
                            mybir.ActivationFunctionType.Copy,
                            scale=cscale,
                        )

                    def compute_in(slab, np_):
                        # input-feature chain, reversed operands: stationary =
                        # X in-cols [k, 16], moving = S^T pair slice -> psum
                        # [16 feat, 256 nodes] covers both blocks of the pair
                        slabC = slab.rearrange("p (kb c) -> p kb c", c=256)
                        pi = ips.tile([CIN, 256], F32, name="pi", tag="pi")
                        for ki in range(NB // 2):
                            nc.tensor.matmul(
                                pi,
                                src3[:, 2 * ki : 2 * ki + 2, CHX:C],
                                slabC[:, 2 * ki : 2 * ki + 2, :],
                                start=(ki == 0),
                                stop=(ki == NB // 2 - 1),
                                perf_mode=DR,
                            )
                        ins8 = ytsp.tile([CIN, 256], F8, name="ins8", tag="ins8")
                        nc.scalar.activation(
                            ins8, pi, mybir.ActivationFunctionType.Copy, scale=cscale
                        )
                        # bf16 spill of both blocks' input rows (one flat DMA;
                        # inb rows r = (m-1)*16 + b*2 + j)
                        insb = ytsp.tile([CIN, 256], BF16, name="insb", tag="insb")
                        nc.gpsimd.tensor_copy(insb, ins8)
                        nc.gpsimd.dma_start(
                            inb[
                                (m - 1) * CIN : m * CIN,
                                np_ * 256 : (np_ + 1) * 256,
                            ],
                            insb,
                        )
                        # orientation fix for the chain: transpose [16, 128]
                        # per block into dst's in-columns
                        for bsel in range(2):
                            nb = 2 * np_ + bsel
                            tpi = tps.tile([128, 64], F8, name="tpi", tag="tpi")
                            tpiv = tpi.rearrange("p (c t) -> p c t", t=2)[
                                :, :CIN, 0:1
                            ]
                            nc.tensor.transpose(
                                tpiv,
                                ins8[:, bsel * 128 : (bsel + 1) * 128],
                                ident8[:CIN, :CIN],
                            )
                            nc.vector.tensor_copy(
                                dst[:, nb * C + CHX : (nb + 1) * C], tpiv
                            )

                    def transpose_block(nb):
                        # transpose the block's hx columns (fp8) into a bf16
                        # staging tile, spill packed by (b, m, u) rows
                        yts = ytsp.tile([128, 512], BF16, name=f"yts{g}", tag="yts")
                        for j in range(4):
                            # fp8 transpose writes one value per 2-byte lane:
                            # output AP must have element step 2
                            tpp = tps.tile([128, 256], F8, name=f"tpp{g}", tag="tpp")
                            tppv = tpp.rearrange("p (c t) -> p c t", t=2)[:, :, 0:1]
                            nc.tensor.transpose(
                                tppv,
                                dst[:, nb * C + j * 128 : nb * C + (j + 1) * 128],
                                ident8,
                            )
                            nc.vector.tensor_copy(
                                yts[:, j * 128 : (j + 1) * 128], tppv
                            )
                        yts4 = yts.rearrange("p (j c) -> p j c", c=128)
                        for bs in range(2):
                            nc.scalar.dma_start(
                                ytbv[bs, :, :, m - 1, nb * 128 : (nb + 1) * 128],
                                yts4[bs * 64 : (bs + 1) * 64],
                            )

                    # transposes deferred by 1 pair so PE never stalls on
                    # the DVE psum-copies feeding them
                    for np_ in range(NB // 2):
                        slab = load_pair(np_)
                        compute_block(slab, np_, 0)
                        compute_block(slab, np_, 1)
                        if g == 0:
                            compute_in(slab, np_)
                        if np_ >= 1:
                            transpose_block(2 * np_ - 2)
                            transpose_block(2 * np_ - 1)
                    transpose_block(NB - 2)
                    transpose_block(NB - 1)

                hop(bufA, bufB, 0, 1, COPY_SCALE_H0)  # y1 = S0 @ y0
                hop(bufB, bufA, 0, 2, COPY_SCALE)  # y2 = S0 @ y1
                hop(bufB, bufA, 1, 3, COPY_SCALE)  # y3 = S1 @ y1
                hop(bufA, bufB, 1, 4, COPY_SCALE)  # y4 = S1 @ y3

        def projection(g):
            D = 128 if g == 0 else 64
            with (
                tc.tile_pool(name=f"ytp{g}", bufs=9) as ytp,
                tc.tile_pool(name=f"aux{g}", bufs=4) as aux,
                tc.tile_pool(name=f"zps{g}", bufs=4, space="PSUM") as zps,
            ):
                for b in range(BLOC):
                    for half in range(NHALF):
                        ns = half * PCH
                        hx_t = aux.tile(
                            [UNITS, PCH], F32, name=f"hx_t{g}", tag="hx_t", bufs=3
                        )
                        nc.sync.dma_start(hx_t, hxt[b, :, ns : ns + PCH])
                        if g == 1:
                            u_t = aux.tile([UNITS, PCH], F32, name="u_t", tag="u_t", bufs=3)
                            nc.gpsimd.dma_start(u_t, u_d[b, :, ns : ns + PCH])
                        # 3 packed rhs tiles: m0ext = [m0 (66) | in-rows m1..4
                        # (8)], p1/p2 = 128 packed hx rows each (m1,m2 | m3,m4)
                        m0e = ytp.tile([74, PCH], BF16, name=f"m0e{g}", tag="m0e")
                        hx_src = (
                            hxtb[b, :, ns : ns + PCH]
                            if g == 0
                            else yt0p[b * UNITS : (b + 1) * UNITS, ns : ns + PCH]
                        )
                        nc.scalar.dma_start(m0e[0:UNITS, :], hx_src)
                        nc.scalar.dma_start(
                            m0e[UNITS:66, :], xint[b * 2 : b * 2 + 2, ns : ns + PCH]
                        )
                        nc.scalar.dma_start(
                            m0e[66:74, :],
                            inb.rearrange("(mm f) n -> mm f n", f=CIN)[
                                :, b * 2 : b * 2 + 2, ns : ns + PCH
                            ],
                        )
                        p1 = ytp.tile([128, PCH], BF16, name=f"p1{g}", tag="p1")
                        nc.sync.dma_start(
                            p1, ytb[g][b * 256 : b * 256 + 128, ns : ns + PCH]
                        )
                        p2 = ytp.tile([128, PCH], BF16, name=f"p2{g}", tag="p2")
                        nc.sync.dma_start(
                            p2, ytb[g][b * 256 + 128 : b * 256 + 256, ns : ns + PCH]
                        )
                        rhs = [m0e, p1, p2]
                        for nfc in range(NFC):
                            zp = zps.tile([D, 512], F32, name=f"zp{g}", tag="zp")
                            for t in range(3):
                                nc.tensor.matmul(
                                    zp,
                                    w_sb[g][t],
                                    rhs[t][:, nfc * 512 : (nfc + 1) * 512],
                                    start=(t == 0),
                                    stop=(t == 2),
                                )
                            nf0 = ns + nfc * 512
                            if g == 0:
                                val = aux.tile([128, 512], F32, name="val", tag="val")
                                nc.scalar.activation(
                                    val,
                                    zp,
                                    mybir.ActivationFunctionType.Sigmoid,
                                    bias=bfn_sb,
                                )
                                rh = aux.tile([64, 512], BF16, name="rh", tag="rh")
                                nc.vector.tensor_mul(
                                    rh,
                                    val[0:64, :],
                                    hx_t[:, nfc * 512 : (nfc + 1) * 512],
                                )
                                nc.gpsimd.dma_start(
                                    u_d[b, :, nf0 : nf0 + 512], val[64:128, :]
                                )
                                nc.gpsimd.dma_start(
                                    yt0p[
                                        b * UNITS : (b + 1) * UNITS, nf0 : nf0 + 512
                                    ],
                                    rh,
                                )
                            else:
                                ct = aux.tile([64, 512], F32, name="ct", tag="ct")
                                nc.scalar.activation(
                                    ct, zp, mybir.ActivationFunctionType.Tanh, bias=bg_sb
                                )
                                tmp = aux.tile([64, 512], F32, name="tmp", tag="tmp")
                                nc.vector.tensor_sub(
                                    tmp, hx_t[:, nfc * 512 : (nfc + 1) * 512], ct
                                )
                                nc.vector.tensor_mul(
                                    tmp, tmp, u_t[:, nfc * 512 : (nfc + 1) * 512]
                                )
                                ot = aux.tile([64, 512], F32, name="ot", tag="ot")
                                nc.vector.tensor_add(ot, tmp, ct)
                                nc.gpsimd.dma_start(outt[b, :, nf0 : nf0 + 512], ot)

        diffusion(0)
        projection(0)
        diffusion(1)
        projection(1)

    nc.compile()
    return nc


def _fold_weights(w, out_dim):
    """w: (330, out). Returns [3, 128, out] bf16: the reference's x0c-mutation
    linear combinations and the fp8 chain storage scale (1/Y_SCALE on
    diffused blocks) folded in, rows packed to match the projection's packed
    rhs tiles: slot 0 = m0 (66) + in-rows of m1..4 (8, zero-padded to 128);
    slot 1 = hx rows of m1,m2; slot 2 = hx rows of m3,m4."""
    Wm = w.reshape(66, 5, out_dim)
    ys = 1.0 / Y_SCALE
    What = np.stack(
        [
            Wm[:, 0] - Wm[:, 2],
            (Wm[:, 1] - Wm[:, 4]) * ys,
            2.0 * ys * Wm[:, 2],
            ys * Wm[:, 3],
            2.0 * ys * Wm[:, 4],
        ]
    )  # [5, 66, out]
    What = np.concatenate([What[:, 2:, :], What[:, :2, :]], axis=1)  # hx rows first
    w0 = np.concatenate([What[0]] + [What[m][64:66] for m in range(1, 5)], axis=0)
    w0 = np.pad(w0, ((0, 128 - w0.shape[0]), (0, 0)))
    w1 = np.concatenate([What[1][0:64], What[2][0:64]], axis=0)
    w2 = np.concatenate([What[3][0:64], What[4][0:64]], axis=0)
    return np.ascontiguousarray(np.stack([w0, w1, w2])).astype(NP_BF16)


_NC_CACHE = {}


def _get_nc(N):
    if N not in _NC_CACHE:
        _NC_CACHE[N] = _build_nc(N)
    return _NC_CACHE[N]


def kernel(inputs, hx, supports, w_fn, b_fn, w_g, b_g):
    inputs = np.ascontiguousarray(np.asarray(inputs), dtype=np.float32)
    hx = np.ascontiguousarray(np.asarray(hx), dtype=np.float32)
    supports = np.ascontiguousarray(np.asarray(supports), dtype=np.float32)
    w_fn = np.asarray(w_fn, dtype=np.float32)
    b_fn = np.asarray(b_fn, dtype=np.float32)
    w_g = np.asarray(w_g, dtype=np.float32)
    b_g = np.asarray(b_g, dtype=np.float32)

    N = supports.shape[1]
    NB = N // 128
    nc = _get_nc(N)

    # ---- replicated tensors ----
    # stb[s, np, kp, kb*256 + j*128 + m] = supports[s][(2np+j)*128+m, kb*128+kp]
    stb = np.ascontiguousarray(
        (supports * np.float32(S_SCALE))
        .reshape(2, NB // 2, 2, 128, NB, 128)
        .transpose(0, 1, 5, 4, 2, 3)
    ).reshape(2, NB // 2, 128, NB * 256).astype(NP_F8)
    wfn_h = _fold_weights(w_fn, 128)
    wg_h = _fold_weights(w_g, 64)
    bfn_h = b_fn.reshape(128, 1).copy()
    bg_h = b_g.reshape(64, 1).copy()

    in_maps = []
    for c in range(NCORES):
        sl = slice(c * BLOC, (c + 1) * BLOC)
        inp_c = inputs[sl].reshape(BLOC, N, IN_DIM)
        hx_c = hx[sl].reshape(BLOC, N, UNITS)
        # X0 [N, 528]: hx cols b*64+u, input cols 512 + b*2 + j
        x0 = np.concatenate(
            [
                hx_c.transpose(1, 0, 2).reshape(N, CHX),
                inp_c.transpose(1, 0, 2).reshape(N, CIN),
            ],
            axis=1,
        )
        x0pm = np.ascontiguousarray(
            x0.reshape(NB, 128, C).transpose(1, 0, 2)
        ).reshape(128, NB * C).astype(NP_F8)
        xin = x0[:, CHX:]
        xint = np.ascontiguousarray(xin.T).astype(NP_BF16)
        hxt = np.ascontiguousarray(hx_c.transpose(0, 2, 1))
        in_maps.append(
            {
                "x0pm": x0pm,
                "stb": stb,
                "xint": xint,
                "hxt": hxt,
                "hxtb": hxt.astype(NP_BF16),
                "wfn": wfn_h,
                "wg": wg_h,
                "bfn": bfn_h,
                "bg": bg_h,
            }
        )

    kernel.last_in_maps = in_maps
    res = run_bass_kernel_spmd(
        nc,
        in_maps,
        core_ids=list(range(NCORES)),
        trace=bool(int(os.environ.get("DCGRU_TRACE", "0"))),
    )

    out = np.empty((B, N * UNITS), np.float32)
    for c in range(NCORES):
        outt = res.results[c]["outt"]  # [BLOC, UNITS, N]
        out[c * BLOC : (c + 1) * BLOC] = outt.transpose(0, 2, 1).reshape(BLOC, -1)
    kernel.last_results = res
    return out


# revision 37
# speedup vs baseline: 1.8117x; 1.8117x over previous
"""DCGRU cell (DCRNN) Trainium2 Bass kernel.

Strategy (see spec sharding_hint): data-parallel over batch B=64 across 8
NeuronCores (8 batches per core); supports + gconv weights replicated.

Math restructuring (validated in numpy against the jax reference):
  reference diffusion xs = [x0, S0@x0, 2*S0^2@x0 - x0, S1@S0@x0, 2*S1^2@S0@x0 - S0@x0]
  -> raw chain     ys = [y0, y1=S0@y0, y2=S0@y1, y3=S1@y1, y4=S1@y3]
  with the 2a-b combinations folded into the projection weights on the host:
  What = [W0-W2, W1-W4, 2*W2, W3, 2*W4] (Wm = rows insz*5+m of the gconv W).

Quantization (validated in numpy: rel err ~2e-4 vs fp32 reference):
  The diffusion chain runs in fp8e4 with MatmulPerfMode.DoubleRow (2
  k-subtiles per matmul, 2x PE throughput vs fp32r/bf16).  S entries are
  ~2.4e-4 (below fp8 subnormal range) so supports are scaled by 2^11 on the
  host; hop outputs are descaled and restored to fp8 with value scale 2^5
  (diffused stds ~0.015) by a fused scaled-copy on the ACT engine.  The 2^-5
  storage scale is folded into the projection weights per diffusion matrix.
  The projection runs in bf16 (weights, YT spills, m=0 operands); the final
  output terms are diluted ~50x relative to the diffusion values, so the
  fp8 chain error contributes only ~2e-4 to the final relative error.

Per-core device layout:
  Diffusion state X [N, 528] fp8 in SBUF, columns c = b*64+u (hx part,
  b=0..7) then 512 + b*2 + j (input part).  Hops are PE DoubleRow matmuls
  out[nb-block, c] += ST_tile[2 k-subtiles].T @ X[2 k-subtiles, c] with
  host-pretransposed, block-packed fp8 supports streamed from HBM.
  After each hop the result is transposed on PE (fp8, 128x128 chunks) and
  spilled to DRAM as YT [528-ish, N] bf16 so the projection can contract
  over features with the feature dim on partitions.  Projection:
  ZT_b[out,n] = sum_m What_m.T @ YT_m[b-rows, n] accumulated in PSUM (bf16
  operands), fused bias+sigmoid/tanh on ACT, gate arithmetic on DVE in
  fp32, all in [units, n] layout; host un-transposes the final output
  during unsharding.
"""

import os
from contextlib import ExitStack

import numpy as np
import ml_dtypes

import concourse.bacc as bacc
import concourse.mybir as mybir
import concourse.tile as tile
from concourse.bass_utils import run_bass_kernel_spmd
from concourse.masks import make_identity

F32 = mybir.dt.float32
F32R = mybir.dt.float32r
BF16 = mybir.dt.bfloat16
F8 = mybir.dt.float8e4
DR = mybir.MatmulPerfMode.DoubleRow

NP_F8 = ml_dtypes.float8_e4m3
NP_BF16 = ml_dtypes.bfloat16

S_SCALE = 2.0**11  # host: supports scaled into fp8 normal range
Y_SCALE = 2.0**5  # stored scale of diffused chain values (stds ~0.015)
# ACT descale on the psum->fp8 hop copy: hop 0 input is unscaled (y0),
# hops >=1 input carries Y_SCALE.
COPY_SCALE_H0 = Y_SCALE / S_SCALE
COPY_SCALE = Y_SCALE / (S_SCALE * Y_SCALE)
# projection weights are pre-scaled by W_SCALE on the host (keeps the fp8
# diffused-weight values in e4m3's normal range); the sigmoid/tanh
# activations descale by 1/W_SCALE.
W_SCALE = 2.0**7
ACT_SCALE = 1.0 / W_SCALE


def _r(ap):
    return ap.bitcast(F32R)

NCORES = 8
B = 64
BLOC = B // NCORES  # 8
IN_DIM = 2
UNITS = 64
CHX = BLOC * UNITS  # 512
C = CHX + BLOC * IN_DIM  # 528
CIN = BLOC * IN_DIM  # 16
CH = C // 2  # 264 (psum free-dim split)


def _build_nc(N):
    """Build the per-core Bass program (SPMD; same NEFF on all 8 cores)."""
    NB = N // 128  # row blocks (32 at full size)
    PCH = min(2048, N)  # phase-P n-chunk held in SBUF
    NHALF = N // PCH
    NFC = PCH // 512  # 512-wide proj chunks per PCH

    nc = bacc.Bacc("TRN2", target_bir_lowering=False, debug=False)

    # ---- external I/O ----
    x0pm = nc.dram_tensor("x0pm", [128, NB * C], F8, kind="ExternalInput").ap()
    # pair-interleaved transposed supports: stb[s, np, kp, kb*256 + j*128 + m]
    # = S[(2*np+j)*128 + m, kb*128 + kp] * S_SCALE
    stb = nc.dram_tensor(
        "stb", [2, NB // 2, 128, NB * 256], F8, kind="ExternalInput"
    ).ap()
    xint = nc.dram_tensor("xint", [CIN, N], BF16, kind="ExternalInput").ap()
    hxt = nc.dram_tensor("hxt", [BLOC, UNITS, N], F32, kind="ExternalInput").ap()
    hxtb = nc.dram_tensor("hxtb", [BLOC, UNITS, N], BF16, kind="ExternalInput").ap()
    wfn = nc.dram_tensor("wfn", [74, 128], BF16, kind="ExternalInput").ap()
    wg = nc.dram_tensor("wg", [74, 64], BF16, kind="ExternalInput").ap()
    w12fn = nc.dram_tensor("w12fn", [128, 2 * 128], F8, kind="ExternalInput").ap()
    w12g = nc.dram_tensor("w12g", [128, 2 * 64], F8, kind="ExternalInput").ap()
    bfn = nc.dram_tensor("bfn", [128, 1], F32, kind="ExternalInput").ap()
    bg = nc.dram_tensor("bg", [64, 1], F32, kind="ExternalInput").ap()
    outt = nc.dram_tensor("outt", [BLOC, UNITS, N], F32, kind="ExternalOutput").ap()

    with tile.TileContext(nc) as tc, ExitStack() as ctx:
        # ---- persistent pools ----
        const = ctx.enter_context(tc.tile_pool(name="const", bufs=1))
        dram = ctx.enter_context(tc.tile_pool(name="dram", bufs=1, space="DRAM"))

        ident8 = const.tile([128, 128], F8, name="ident8")
        make_identity(nc, ident8)
        # packed projection weights per gconv (pre-scaled by W_SCALE):
        # w0 (bf16) = [m0 rows (66) + in-rows of m1..4 (8)] = 74 rows;
        # w12 (fp8) = [128, 2, D]: k-subtile 0 = hx rows of m1,m2, k-subtile
        # 1 = m3,m4 -- one DoubleRow matmul against the packed ytb rhs.
        w_sb = {}
        for g, wsrc, w12src, D in ((0, wfn, w12fn, 128), (1, wg, w12g, 64)):
            w0t = const.tile([74, D], BF16, name=f"w0_{g}")
            nc.sync.dma_start(w0t, wsrc)
            w12t = const.tile([128, 2 * D], F8, name=f"w12_{g}")
            nc.sync.dma_start(w12t, w12src)
            w_sb[g] = (w0t, w12t.rearrange("p (j d) -> p j d", j=2))
        bfn_sb = const.tile([128, 1], F32, name="bfn_sb")
        nc.sync.dma_start(bfn_sb, bfn)
        bg_sb = const.tile([64, 1], F32, name="bg_sb")
        nc.sync.dma_start(bg_sb, bg)
        # DRAM scratch (bf16): per-gconv packed transposed diffusion results.
        # ytb[g] rows r = b*256 + (m-1)*64 + u (hx rows of hops m=1..4, packed
        # so the projection loads 128-row slabs); inb rows r = b*8 + (m-1)*2
        # + j (input-feature rows of hops m=1..4, shared by both gconvs).
        if os.environ.get("DCGRU_DEBUG", "0") == "1":
            ytb = [
                nc.dram_tensor(f"ytb{g}", [BLOC * 256, N], F8, kind="ExternalOutput").ap()
                for g in range(2)
            ]
            inb = nc.dram_tensor("inb", [4 * CIN, N], BF16, kind="ExternalOutput").ap()
            yt0p = nc.dram_tensor("yt0p", [CHX, N], BF16, kind="ExternalOutput").ap()
        else:
            ytb = [
                dram.tile([BLOC * 256, N], F8, name=f"ytb{g}", tag=f"ytb{g}")
                for g in range(2)
            ]
            inb = dram.tile([4 * CIN, N], BF16, name="inb", tag="inb")
            yt0p = dram.tile([CHX, N], BF16, name="yt0p", tag="yt0p")
        u_d = dram.tile([BLOC, UNITS, N], F32, name="u_d", tag="u_d")

        def diffusion(g):
            """4 hops; X0 loaded from DRAM (x0pm for g=0, yt0p^T for g=1)."""
            with (
                tc.tile_pool(name=f"ybuf{g}", bufs=1) as yp,
                tc.tile_pool(name=f"st{g}", bufs=2) as stp,
                tc.tile_pool(name=f"dps{g}", bufs=2, space="PSUM") as dps,
                tc.tile_pool(name=f"ips{g}", bufs=2, space="PSUM") as ips,
                tc.tile_pool(name=f"tps{g}", bufs=2, space="PSUM") as tps,
                tc.tile_pool(name=f"yts{g}", bufs=3) as ytsp,
            ):
                bufA = yp.tile([128, NB * C], F8, name=f"bufA{g}", tag="bufA")
                bufB = yp.tile([128, NB * C], F8, name=f"bufB{g}", tag="bufB")
                if g == 0:
                    q4 = NB * C // 4
                    for q in range(4):
                        nc.sync.dma_start(
                            bufA[:, q * q4 : (q + 1) * q4],
                            x0pm[:, q * q4 : (q + 1) * q4],
                        )
                else:
                    # x0' = r*hx lives transposed in yt0p [512, N]; XBAR
                    # DMA-transpose each node block then cast bf16 -> fp8
                    for kb in range(NB):
                        xb = ytsp.tile([128, CHX], BF16, name="xb", tag="xb")
                        nc.sync.dma_start_transpose(
                            xb, yt0p[:, kb * 128 : (kb + 1) * 128]
                        )
                        nc.vector.tensor_copy(
                            bufA[:, kb * C : kb * C + CHX], xb
                        )

                # gconv2 skips the 16 input columns entirely: their diffusion
                # is identical to gconv1's, so phase P reuses g1's spills.
                # packed spill views (see ytb/inb row layout comments)
                ytbv = ytb[g].rearrange(
                    "(jj bs mm u) n -> bs u jj mm n", jj=4, bs=2, mm=4, u=64
                )

                def hop(src, dst, s_idx, m, cscale):
                    src3 = src.rearrange("p (k c) -> p k c", c=C)

                    def load_pair(np_):
                        slab = stp.tile(
                            [128, NB * 256], F8, name=f"slab{g}", tag="slab"
                        )
                        nc.sync.dma_start(slab, stb[s_idx, np_])
                        return slab

                    def compute_block(slab, np_, bsel):
                        # main hx chain: single 512-wide psum, 16 DoubleRow
                        # matmuls (2 k-subtiles each)
                        nb = 2 * np_ + bsel
                        slabM = slab.rearrange(
                            "p (kb j c) -> p kb j c", j=2, c=128
                        )
                        pa = dps.tile([128, CHX], F32, name=f"pa{g}", tag="pa")
                        for ki in range(NB // 2):
                            nc.tensor.matmul(
                                pa,
                                slabM[:, 2 * ki : 2 * ki + 2, bsel, :],
                                src3[:, 2 * ki : 2 * ki + 2, 0:CHX],
                                start=(ki == 0),
                                stop=(ki == NB // 2 - 1),
                                perf_mode=DR,
                            )
                        # fused descale + fp32->fp8 store on ACT
                        nc.scalar.activation(
                            dst[:, nb * C : nb * C + CHX],
                            pa,
                            mybir.ActivationFunctionType.Copy,
                            scale=cscale,
                        )

                    def compute_in(slab, np_):
                        # input-feature chain, reversed operands: stationary =
                        # X in-cols [k, 16], moving = S^T pair slice -> psum
                        # [16 feat, 256 nodes] covers both blocks of the pair
                        slabC = slab.rearrange("p (kb c) -> p kb c", c=256)
                        pi = ips.tile([CIN, 256], F32, name="pi", tag="pi")
                        for ki in range(NB // 2):
                            nc.tensor.matmul(
                                pi,
                                src3[:, 2 * ki : 2 * ki + 2, CHX:C],
                                slabC[:, 2 * ki : 2 * ki + 2, :],
                                start=(ki == 0),
                                stop=(ki == NB // 2 - 1),
                                perf_mode=DR,
                            )
                        ins8 = ytsp.tile([CIN, 256], F8, name="ins8", tag="ins8")
                        nc.scalar.activation(
                            ins8, pi, mybir.ActivationFunctionType.Copy, scale=cscale
                        )
                        # bf16 spill of both blocks' input rows (one flat DMA;
                        # inb rows r = (m-1)*16 + b*2 + j)
                        insb = ytsp.tile([CIN, 256], BF16, name="insb", tag="insb")
                        nc.gpsimd.tensor_copy(insb, ins8)
                        nc.gpsimd.dma_start(
                            inb[
                                (m - 1) * CIN : m * CIN,
                                np_ * 256 : (np_ + 1) * 256,
                            ],
                            insb,
                        )
                        # orientation fix for the chain: transpose [16, 128]
                        # per block into dst's in-columns
                        for bsel in range(2):
                            nb = 2 * np_ + bsel
                            tpi = tps.tile([128, 64], F8, name="tpi", tag="tpi")
                            tpiv = tpi.rearrange("p (c t) -> p c t", t=2)[
                                :, :CIN, 0:1
                            ]
                            nc.tensor.transpose(
                                tpiv,
                                ins8[:, bsel * 128 : (bsel + 1) * 128],
                                ident8[:CIN, :CIN],
                            )
                            nc.vector.tensor_copy(
                                dst[:, nb * C + CHX : (nb + 1) * C], tpiv
                            )

                    def transpose_block(nb):
                        # transpose the block's hx columns into an fp8
                        # staging tile, spill packed by (b, m, u) rows
                        yts = ytsp.tile([128, 512], F8, name=f"yts{g}", tag="yts")
                        for j in range(4):
                            # fp8 transpose writes one value per 2-byte lane:
                            # output AP must have element step 2
                            tpp = tps.tile([128, 256], F8, name=f"tpp{g}", tag="tpp")
                            tppv = tpp.rearrange("p (c t) -> p c t", t=2)[:, :, 0:1]
                            nc.tensor.transpose(
                                tppv,
                                dst[:, nb * C + j * 128 : nb * C + (j + 1) * 128],
                                ident8,
                            )
                            nc.vector.tensor_copy(
                                yts[:, j * 128 : (j + 1) * 128], tppv
                            )
                        yts4 = yts.rearrange("p (j c) -> p j c", c=128)
                        for bs in range(2):
                            nc.scalar.dma_start(
                                ytbv[bs, :, :, m - 1, nb * 128 : (nb + 1) * 128],
                                yts4[bs * 64 : (bs + 1) * 64],
                            )

                    # transposes deferred by 1 pair so PE never stalls on
                    # the DVE psum-copies feeding them
                    for np_ in range(NB // 2):
                        slab = load_pair(np_)
                        compute_block(slab, np_, 0)
                        compute_block(slab, np_, 1)
                        if g == 0:
                            compute_in(slab, np_)
                        if np_ >= 1:
                            transpose_block(2 * np_ - 2)
                            transpose_block(2 * np_ - 1)
                    transpose_block(NB - 2)
                    transpose_block(NB - 1)

                hop(bufA, bufB, 0, 1, COPY_SCALE_H0)  # y1 = S0 @ y0
                hop(bufB, bufA, 0, 2, COPY_SCALE)  # y2 = S0 @ y1
                hop(bufB, bufA, 1, 3, COPY_SCALE)  # y3 = S1 @ y1
                hop(bufA, bufB, 1, 4, COPY_SCALE)  # y4 = S1 @ y3

        def projection(g):
            D = 128 if g == 0 else 64
            with (
                tc.tile_pool(name=f"ytp{g}", bufs=9) as ytp,
                tc.tile_pool(name=f"aux{g}", bufs=4) as aux,
                tc.tile_pool(name=f"zps{g}", bufs=4, space="PSUM") as zps,
            ):
                for half in range(NHALF):
                    for b in range(BLOC):
                        ns = half * PCH
                        if g == 1:
                            hx_t = aux.tile(
                                [UNITS, PCH], F32, name="hx_t", tag="hx_t", bufs=3
                            )
                            nc.sync.dma_start(hx_t, hxt[b, :, ns : ns + PCH])
                            u_t = aux.tile(
                                [UNITS, PCH], BF16, name="u_t", tag="u_t", bufs=3
                            )
                            nc.gpsimd.dma_start(u_t, u_d[b, :, ns : ns + PCH])
                        # packed rhs: m0e = [m0 (66) | in-rows m1..4 (8)]
                        # (bf16); p12 = [128, 2, PCH] fp8, k-subtile 0 =
                        # m1,m2 hx rows, 1 = m3,m4 -> one DoubleRow matmul
                        m0e = ytp.tile([74, PCH], BF16, name=f"m0e{g}", tag="m0e")
                        hx_src = (
                            hxtb[b, :, ns : ns + PCH]
                            if g == 0
                            else yt0p[b * UNITS : (b + 1) * UNITS, ns : ns + PCH]
                        )
                        nc.scalar.dma_start(m0e[0:UNITS, :], hx_src)
                        nc.scalar.dma_start(
                            m0e[UNITS:66, :], xint[b * 2 : b * 2 + 2, ns : ns + PCH]
                        )
                        nc.scalar.dma_start(
                            m0e[66:74, :],
                            inb.rearrange("(mm f) n -> mm f n", f=CIN)[
                                :, b * 2 : b * 2 + 2, ns : ns + PCH
                            ],
                        )
                        p12 = ytp.tile([128, 2, PCH], F8, name=f"p12{g}", tag="p12")
                        nc.sync.dma_start(
                            p12[:, 0:1, :],
                            ytb[g][b * 256 : b * 256 + 128, ns : ns + PCH],
                        )
                        nc.sync.dma_start(
                            p12[:, 1:2, :],
                            ytb[g][b * 256 + 128 : b * 256 + 256, ns : ns + PCH],
                        )
                        for nfc in range(NFC):
                            zp = zps.tile([D, 512], F32, name=f"zp{g}", tag="zp")
                            nc.tensor.matmul(
                                zp,
                                w_sb[g][0],
                                m0e[:, nfc * 512 : (nfc + 1) * 512],
                                start=True,
                                stop=False,
                            )
                            nc.tensor.matmul(
                                zp,
                                w_sb[g][1],
                                p12[:, :, nfc * 512 : (nfc + 1) * 512],
                                start=False,
                                stop=True,
                                perf_mode=DR,
                            )
                            nf0 = ns + nfc * 512
                            if g == 0:
                                val = aux.tile([128, 512], BF16, name="val", tag="val")
                                nc.scalar.activation(
                                    val,
                                    zp,
                                    mybir.ActivationFunctionType.Sigmoid,
                                    bias=bfn_sb,
                                    scale=ACT_SCALE,
                                )
                                rh = aux.tile([64, 512], BF16, name="rh", tag="rh")
                                nc.vector.tensor_mul(
                                    rh,
                                    val[0:64, :],
                                    m0e[0:UNITS, nfc * 512 : (nfc + 1) * 512],
                                )
                                nc.gpsimd.dma_start(
                                    u_d[b, :, nf0 : nf0 + 512], val[64:128, :]
                                )
                                nc.gpsimd.dma_start(
                                    yt0p[
                                        b * UNITS : (b + 1) * UNITS, nf0 : nf0 + 512
                                    ],
                                    rh,
                                )
                            else:
                                ct = aux.tile([64, 512], F32, name="ct", tag="ct")
                                nc.scalar.activation(
                                    ct,
                                    zp,
                                    mybir.ActivationFunctionType.Tanh,
                                    bias=bg_sb,
                                    scale=ACT_SCALE,
                                )
                                tmp = aux.tile([64, 512], F32, name="tmp", tag="tmp")
                                nc.vector.tensor_sub(
                                    tmp, hx_t[:, nfc * 512 : (nfc + 1) * 512], ct
                                )
                                nc.vector.tensor_mul(
                                    tmp, tmp, u_t[:, nfc * 512 : (nfc + 1) * 512]
                                )
                                ot = aux.tile([64, 512], F32, name="ot", tag="ot")
                                nc.vector.tensor_add(ot, tmp, ct)
                                nc.gpsimd.dma_start(outt[b, :, nf0 : nf0 + 512], ot)

        diffusion(0)
        projection(0)
        diffusion(1)
        projection(1)

    nc.compile()
    return nc


def _fold_weights(w, out_dim):
    """w: (330, out). Returns (w0 [74, out] bf16, w12 [128, 2*out] fp8): the
    reference's x0c-mutation linear combinations, the fp8 chain storage scale
    (1/Y_SCALE on diffused blocks), and the global W_SCALE pre-scale folded
    in.  Rows packed to match the projection's rhs tiles: w0 = m0 (66) +
    in-rows of m1..4 (8); w12 k-subtile 0 = hx rows of m1,m2, 1 = m3,m4."""
    Wm = w.reshape(66, 5, out_dim)
    ys = 1.0 / Y_SCALE
    What = np.stack(
        [
            Wm[:, 0] - Wm[:, 2],
            (Wm[:, 1] - Wm[:, 4]) * ys,
            2.0 * ys * Wm[:, 2],
            ys * Wm[:, 3],
            2.0 * ys * Wm[:, 4],
        ]
    ) * np.float32(W_SCALE)  # [5, 66, out]
    What = np.concatenate([What[:, 2:, :], What[:, :2, :]], axis=1)  # hx rows first
    w0 = np.concatenate([What[0]] + [What[m][64:66] for m in range(1, 5)], axis=0)
    w1 = np.concatenate([What[1][0:64], What[2][0:64]], axis=0)
    w2 = np.concatenate([What[3][0:64], What[4][0:64]], axis=0)
    w12 = np.stack([w1, w2], axis=1).reshape(128, 2 * out_dim)
    return (
        np.ascontiguousarray(w0).astype(NP_BF16),
        np.ascontiguousarray(w12).astype(NP_F8),
    )


_NC_CACHE = {}


def _get_nc(N):
    if N not in _NC_CACHE:
        _NC_CACHE[N] = _build_nc(N)
    return _NC_CACHE[N]


def kernel(inputs, hx, supports, w_fn, b_fn, w_g, b_g):
    inputs = np.ascontiguousarray(np.asarray(inputs), dtype=np.float32)
    hx = np.ascontiguousarray(np.asarray(hx), dtype=np.float32)
    supports = np.ascontiguousarray(np.asarray(supports), dtype=np.float32)
    w_fn = np.asarray(w_fn, dtype=np.float32)
    b_fn = np.asarray(b_fn, dtype=np.float32)
    w_g = np.asarray(w_g, dtype=np.float32)
    b_g = np.asarray(b_g, dtype=np.float32)

    N = supports.shape[1]
    NB = N // 128
    nc = _get_nc(N)

    # ---- replicated tensors ----
    # stb[s, np, kp, kb*256 + j*128 + m] = supports[s][(2np+j)*128+m, kb*128+kp]
    stb = np.ascontiguousarray(
        (supports * np.float32(S_SCALE))
        .reshape(2, NB // 2, 2, 128, NB, 128)
        .transpose(0, 1, 5, 4, 2, 3)
    ).reshape(2, NB // 2, 128, NB * 256).astype(NP_F8)
    wfn_h, w12fn_h = _fold_weights(w_fn, 128)
    wg_h, w12g_h = _fold_weights(w_g, 64)
    bfn_h = b_fn.reshape(128, 1).copy()
    bg_h = b_g.reshape(64, 1).copy()

    in_maps = []
    for c in range(NCORES):
        sl = slice(c * BLOC, (c + 1) * BLOC)
        inp_c = inputs[sl].reshape(BLOC, N, IN_DIM)
        hx_c = hx[sl].reshape(BLOC, N, UNITS)
        # X0 [N, 528]: hx cols b*64+u, input cols 512 + b*2 + j
        x0 = np.concatenate(
            [
                hx_c.transpose(1, 0, 2).reshape(N, CHX),
                inp_c.transpose(1, 0, 2).reshape(N, CIN),
            ],
            axis=1,
        )
        x0pm = np.ascontiguousarray(
            x0.reshape(NB, 128, C).transpose(1, 0, 2)
        ).reshape(128, NB * C).astype(NP_F8)
        xin = x0[:, CHX:]
        xint = np.ascontiguousarray(xin.T).astype(NP_BF16)
        hxt = np.ascontiguousarray(hx_c.transpose(0, 2, 1))
        in_maps.append(
            {
                "x0pm": x0pm,
                "stb": stb,
                "xint": xint,
                "hxt": hxt,
                "hxtb": hxt.astype(NP_BF16),
                "wfn": wfn_h,
                "wg": wg_h,
                "w12fn": w12fn_h,
                "w12g": w12g_h,
                "bfn": bfn_h,
                "bg": bg_h,
            }
        )

    kernel.last_in_maps = in_maps
    res = run_bass_kernel_spmd(
        nc,
        in_maps,
        core_ids=list(range(NCORES)),
        trace=bool(int(os.environ.get("DCGRU_TRACE", "0"))),
    )

    out = np.empty((B, N * UNITS), np.float32)
    for c in range(NCORES):
        outt = res.results[c]["outt"]  # [BLOC, UNITS, N]
        out[c * BLOC : (c + 1) * BLOC] = outt.transpose(0, 2, 1).reshape(BLOC, -1)
    kernel.last_results = res
    return out
